# revision 18
# baseline (speedup 1.0000x reference)
"""Causal attention (B=8, N=4096 flattened 64x64, d=128) on 8 trn2 cores.

Sharding: data-parallel over batch -- core b gets batch element b.

Per-core algorithm (flash-style, transposed orientation):
  inputs per core (host pre-transposed):
    qT [128, 4096] bf16  (c on partitions, query pos on free)
    kT [128, 4096] bf16
    v  [4096, 128] bf16 (natural; loaded as [128,128] tiles)
  loop q-chunks of 512 (t = 0..7), k-tiles of 128 (j = 0..4t+3):
    S^T[k, q] = kT_j.T @ qT_chunk          (PE, PSUM, N=512, bf16 moving)
    E = exp(S^T / sqrt(128))  -> bf16      (ScalarE, PSUM->SBUF, groups of 3 j)
    causal mask on diagonal tiles          (DVE multiply by [0-prefix|triangle])
    O^T += v_j.T @ E_j                     (PE, accumulate in PSUM over j)
    denom[q] += sum_k E_j[k, q]            (split: PE all-ones matmul / DVE adds)
  Diagonal k-tiles narrow their S/PV matmuls to the non-masked column range.
  outputs per core: outT [128, 4096] (unnormalized O^T), den [1, 4096]
  host: out = (outT / den).T

All matmul operands are bf16 (1 PE cycle/column, FWL weight loads; walrus
rejects mixed 16/32-bit matmul inputs). exp output E is bf16: PV weights are
a softmax of slightly perturbed logits, so normalization stays consistent.

No max-subtraction in softmax: scores are ~N(0,1) (max |s| < ~7), exp is safe
in fp32 and softmax is shift-invariant. Masked probabilities are exactly zero
(multiplicative mask), matching the reference's `softmax(.)*allowed`.
"""

import math

import ml_dtypes
import numpy as np

import concourse.bacc as bacc
import concourse.mybir as mybir
import concourse.tile as tile
from concourse.bass import ts, ds
from concourse.bass_utils import run_bass_kernel_spmd

P = 128
NSEQ = 4096
QCH = 512              # query positions per chunk
NCH = NSEQ // QCH      # 8 chunks
GROUP = 3              # k-tiles per exp group (3 PSUM banks; x2 buffered)
NPIECE = 8             # input DMA pieces per tensor
SCALE = 1.0 / math.sqrt(128.0)
F32 = mybir.dt.float32
F32R = mybir.dt.float32r  # single-pass PE mode: 1 cyc/col vs fp32's 4
BF16 = mybir.dt.bfloat16
N_CORES = 8
PE_DEN_MOD = 2         # j % PE_DEN_MOD == 1 -> denominator on PE

_nc_cache = []


def _round_fp32r(x):
    """Round fp32 array to fp32r (11-bit mantissa, round-nearest-even)."""
    b = np.ascontiguousarray(x, np.float32).view(np.uint32)
    bias = np.uint32(0x7FF) + ((b >> np.uint32(12)) & np.uint32(1))
    b = (b + bias) & np.uint32(0xFFFFF000)
    return b.view(np.float32)


def _build():
    nc = bacc.Bacc("TRN2", target_bir_lowering=False, debug=False,
                   num_devices=N_CORES)
    qT = nc.dram_tensor("qT", [P, NSEQ], BF16, kind="ExternalInput").ap()
    kT = nc.dram_tensor("kT", [P, NSEQ], BF16, kind="ExternalInput").ap()
    v = nc.dram_tensor("v", [NSEQ, P], BF16, kind="ExternalInput").ap()
    outT = nc.dram_tensor("outT", [P, NSEQ], F32, kind="ExternalOutput").ap()
    den = nc.dram_tensor("den", [1, NSEQ], F32, kind="ExternalOutput").ap()

    exp_fn = mybir.ActivationFunctionType.Exp

    with tile.TileContext(nc) as tc:
        with (
            tc.tile_pool(name="const", bufs=1) as cpool,
            tc.tile_pool(name="epool", bufs=13) as epool,
            tc.tile_pool(name="qpool", bufs=12) as qpool,
            tc.tile_pool(name="spool", bufs=2) as spool,
            tc.tile_pool(name="ps_s", bufs=2, space="PSUM") as ps_pool,
            tc.tile_pool(name="ps_o", bufs=1, space="PSUM") as po_pool,
            tc.tile_pool(name="ps_d", bufs=1, space="PSUM") as pd_pool,
        ):
            # constants built on fp32 scratch (memset/affine_select can't
            # write fp32r), then round-copied into typed tiles
            ones_sq = cpool.tile([P, P], BF16)
            nc.gpsimd.memset(ones_sq, 1.0)
            # pre-warm the PE during the input-DMA wait (~4us of dummy
            # matmuls) so the HAM clock gate is at 2.4 GHz for real work;
            # chunk 0's first denominator matmul clears the db bank anyway
            warm_db = pd_pool.tile([P, QCH], F32, tag="db", name="warm")
            for wi in range(32):
                nc.tensor.matmul(warm_db[:, ds(0, 64)], ones_sq,
                                 ones_sq[:, :64], start=True, stop=True)
            scratch = cpool.tile([P, 4 * P], F32)
            # pmask_d [128, (d+1)*128]: zeros prefix then upper-triangle
            # (keep where c - r - 128*d >= 0); multiplicative causal mask
            pmasks = []
            for d in range(4):
                w = (d + 1) * P
                nc.gpsimd.memset(scratch[:, :w], 1.0)
                nc.gpsimd.affine_select(
                    out=scratch[:, :w], in_=scratch[:, :w],
                    compare_op=mybir.AluOpType.is_ge, fill=0.0,
                    base=-d * P, pattern=[[1, w]], channel_multiplier=-1)
                pm = cpool.tile([P, w], BF16, name=f"pmask{d}")
                nc.vector.tensor_copy(pm, scratch[:, :w])
                pmasks.append(pm)

            # zero the two rotating S-PSUM slots once: narrowed diagonal
            # S-matmuls never write their masked column prefix, and
            # exp(uninitialized PSUM) can be inf (inf * 0 mask -> NaN)
            for si in range(2):
                s_init = ps_pool.tile([P, GROUP * QCH], F32, tag="s",
                                      name=f"s_init{si}")
                nc.vector.memset(s_init, 0.0)

            qT_sb = cpool.tile([P, NSEQ], BF16)
            kT_sb = cpool.tile([P, NSEQ], BF16)
            v_sb = cpool.tile([P, NSEQ], BF16)
            pw = NSEQ // NPIECE      # columns / k-rows per DMA piece
            for pi in range(NPIECE):
                sl = ds(pi * pw, pw)
                kq = nc.scalar if pi == 0 else nc.sync
                kq.dma_start(kT_sb[:, sl], kT[:, sl])
                nc.sync.dma_start(qT_sb[:, sl], qT[:, sl])
                vq = nc.gpsimd if pi == 0 else nc.sync
                vq.dma_start(
                    v_sb[:, sl].rearrange("p (j c) -> p j c", c=P),
                    v[sl, :].rearrange("(j p) c -> p j c", p=P))

            def emit_pv(job):
                # deferred PV + denominator matmuls for one group
                # (software pipelining: keeps the in-order PE queue from
                # head-of-line blocking on the exp/mask chain of the group)
                (t, j0, gn, nj, e_sb, o_ps, db_ps, den_blk,
                 den_first, den_last) = job
                for d in range(gn):
                    j = j0 + d
                    dd = j - 4 * t
                    off = max(dd, 0) * P
                    nc.tensor.matmul(
                        o_ps[:, ds(off, QCH - off)],
                        v_sb[:, ts(j, P)],
                        e_sb[:, ds(d * QCH + off, QCH - off)],
                        start=(j == 0), stop=(j == nj - 1))
                if den_blk is not None:
                    nc.tensor.matmul(db_ps, ones_sq, den_blk,
                                     start=den_first, stop=den_last)
                if j0 + gn == nj:      # last group: flush chunk outputs
                    out_sb = spool.tile([P, QCH], F32, tag="osb",
                                        name=f"osb{t}")
                    den_sb = spool.tile([1, QCH], F32, tag="den",
                                        name=f"den{t}")
                    if t == NCH - 1:   # tail: split copies across engines
                        nc.scalar.copy(out_sb, o_ps)
                        nc.vector.tensor_copy(den_sb, db_ps[0:1, :])
                    else:
                        nc.vector.tensor_copy(out_sb, o_ps)
                        nc.vector.tensor_copy(den_sb, db_ps[0:1, :])
                    nc.sync.dma_start(outT[:, ts(t, QCH)], out_sb)
                    nc.sync.dma_start(den[:, ts(t, QCH)], den_sb)

            pv_pending = None
            for t in range(NCH):
                nj = 4 * (t + 1)          # causal: k-tiles 0..4t+3
                q_sl = qT_sb[:, ts(t, QCH)]
                o_ps = po_pool.tile([P, QCH], F32, tag="o")
                db_ps = pd_pool.tile([P, QCH], F32, tag="db")
                den_carry = None
                den_count = 0

                groups = []
                j0 = 0
                while j0 < nj:
                    gn = min(GROUP, nj - j0)
                    groups.append((j0, gn))
                    j0 += gn

                for (j0, gn) in groups:
                    s_ps = ps_pool.tile([P, gn * QCH], F32, tag="s",
                                        padded_shape=[P, GROUP * QCH])
                    for d in range(gn):
                        j = j0 + d
                        dd = j - 4 * t
                        off = max(dd, 0) * P   # fully-masked column prefix
                        nc.tensor.matmul(
                            s_ps[:, ds(d * QCH + off, QCH - off)],
                            kT_sb[:, ts(j, P)], q_sl[:, ds(off, QCH - off)],
                            start=True, stop=True)
                    e_sb = epool.tile([P, gn * QCH], BF16, tag="e",
                                      padded_shape=[P, GROUP * QCH])
                    nc.scalar.activation(e_sb, s_ps, exp_fn, scale=SCALE)

                    # causal mask on diagonal tiles (j in [4t, 4t+4)):
                    # one multiply with the [zeros-prefix | triangle] mask
                    for d in range(gn):
                        j = j0 + d
                        dd = j - 4 * t
                        if dd >= 0:
                            reg = e_sb[:, ds(d * QCH, (dd + 1) * P)]
                            nc.vector.tensor_mul(reg, reg, pmasks[dd])

                    # denominator partials: sum blocks on DVE (bf16 2x
                    # adds), chaining across pairs of groups; one all-ones
                    # matmul per pair reduces over partitions into db
                    gidx = j0 // GROUP
                    chain = den_carry if gidx % 2 == 1 else None
                    if gn == 1 and chain is None:
                        den_blk = e_sb[:, :QCH]
                    else:
                        qacc = qpool.tile([P, QCH], BF16, tag="qacc")
                        first2 = (chain if chain is not None
                                  else e_sb[:, ts(1, QCH)])
                        nc.vector.tensor_add(qacc, e_sb[:, ts(0, QCH)],
                                             first2)
                        for d in range(1 if chain is not None else 2, gn):
                            nc.vector.tensor_add(qacc, qacc,
                                                 e_sb[:, ts(d, QCH)])
                        den_blk = qacc
                    if gidx % 2 == 0 and j0 + gn < nj:
                        den_carry = den_blk      # defer to next group
                        den_blk = None
                    else:
                        den_carry = None

                    if pv_pending is not None:
                        emit_pv(pv_pending)
                    den_first = den_blk is not None and den_count == 0
                    den_last = j0 + gn == nj
                    if den_blk is not None:
                        den_count += 1
                    pv_pending = (t, j0, gn, nj, e_sb, o_ps, db_ps, den_blk,
                                  den_first, den_last)

            emit_pv(pv_pending)

    nc.compile()
    return nc


def _get_nc():
    if not _nc_cache:
        _nc_cache.append(_build())
    return _nc_cache[0]


def kernel(query, key, value):
    B, H, W, C = query.shape
    CV = value.shape[-1]
    n = H * W
    q = (np.asarray(query, np.float32).reshape(B, n, C).transpose(0, 2, 1)
         .astype(ml_dtypes.bfloat16))
    q = np.ascontiguousarray(q)
    k = np.ascontiguousarray(
        np.asarray(key, np.float32).reshape(B, n, C).transpose(0, 2, 1)
        .astype(ml_dtypes.bfloat16))
    v = np.ascontiguousarray(
        np.asarray(value, np.float32).reshape(B, n, CV)
        .astype(ml_dtypes.bfloat16))

    nc = _get_nc()
    in_maps = [{"qT": q[b], "kT": k[b], "v": v[b]} for b in range(B)]
    res = run_bass_kernel_spmd(nc, in_maps, core_ids=list(range(N_CORES)))

    out = np.empty((B, n, CV), np.float32)
    for b in range(B):
        oT = res.results[b]["outT"]          # [128, 4096] unnormalized O^T
        dn = res.results[b]["den"]           # [1, 4096]
        out[b] = (oT / dn).T
    return out.reshape(B, H, W, CV)


# revision 19
# speedup vs baseline: 1.0048x; 1.0048x over previous
"""Causal attention (B=8, N=4096 flattened 64x64, d=128) on 8 trn2 cores.

Sharding: data-parallel over batch -- core b gets batch element b.

Per-core algorithm (flash-style, transposed orientation):
  inputs per core (host pre-transposed):
    qT [128, 4096] bf16  (c on partitions, query pos on free)
    kT [128, 4096] bf16
    v  [4096, 128] bf16 (natural; loaded as [128,128] tiles)
  loop q-chunks of 512 (t = 0..7), k-tiles of 128 (j = 0..4t+3):
    S^T[k, q] = kT_j.T @ qT_chunk          (PE, PSUM, N=512, bf16 moving)
    E = exp(S^T / sqrt(128))  -> bf16      (ScalarE, PSUM->SBUF, groups of 3 j)
    causal mask on diagonal tiles          (DVE multiply by [0-prefix|triangle])
    O^T += v_j.T @ E_j                     (PE, accumulate in PSUM over j)
    denom[q] += sum_k E_j[k, q]            (split: PE all-ones matmul / DVE adds)
  Diagonal k-tiles narrow their S/PV matmuls to the non-masked column range.
  outputs per core: outT [128, 4096] (unnormalized O^T), den [1, 4096]
  host: out = (outT / den).T

All matmul operands are bf16 (1 PE cycle/column, FWL weight loads; walrus
rejects mixed 16/32-bit matmul inputs). exp output E is bf16: PV weights are
a softmax of slightly perturbed logits, so normalization stays consistent.

No max-subtraction in softmax: scores are ~N(0,1) (max |s| < ~7), exp is safe
in fp32 and softmax is shift-invariant. Masked probabilities are exactly zero
(multiplicative mask), matching the reference's `softmax(.)*allowed`.
"""

import math

import ml_dtypes
import numpy as np

import concourse.bacc as bacc
import concourse.mybir as mybir
import concourse.tile as tile
from concourse.bass import ts, ds
from concourse.bass_utils import run_bass_kernel_spmd

P = 128
NSEQ = 4096
QCH = 512              # query positions per chunk
NCH = NSEQ // QCH      # 8 chunks
GROUP = 3              # k-tiles per exp group (3 PSUM banks; x2 buffered)
NPIECE = 8             # input DMA pieces per tensor
SCALE = 1.0 / math.sqrt(128.0)
F32 = mybir.dt.float32
F32R = mybir.dt.float32r  # single-pass PE mode: 1 cyc/col vs fp32's 4
BF16 = mybir.dt.bfloat16
N_CORES = 8
PE_DEN_MOD = 2         # j % PE_DEN_MOD == 1 -> denominator on PE

_nc_cache = []


def _round_fp32r(x):
    """Round fp32 array to fp32r (11-bit mantissa, round-nearest-even)."""
    b = np.ascontiguousarray(x, np.float32).view(np.uint32)
    bias = np.uint32(0x7FF) + ((b >> np.uint32(12)) & np.uint32(1))
    b = (b + bias) & np.uint32(0xFFFFF000)
    return b.view(np.float32)


def _build():
    nc = bacc.Bacc("TRN2", target_bir_lowering=False, debug=False,
                   num_devices=N_CORES)
    qT = nc.dram_tensor("qT", [P, NSEQ], BF16, kind="ExternalInput").ap()
    kT = nc.dram_tensor("kT", [P, NSEQ], BF16, kind="ExternalInput").ap()
    v = nc.dram_tensor("v", [NSEQ, P], BF16, kind="ExternalInput").ap()
    outT = nc.dram_tensor("outT", [P, NSEQ], F32, kind="ExternalOutput").ap()
    den = nc.dram_tensor("den", [1, NSEQ], F32, kind="ExternalOutput").ap()

    exp_fn = mybir.ActivationFunctionType.Exp

    with tile.TileContext(nc) as tc:
        with (
            tc.tile_pool(name="const", bufs=1) as cpool,
            tc.tile_pool(name="epool", bufs=13) as epool,
            tc.tile_pool(name="qpool", bufs=12) as qpool,
            tc.tile_pool(name="spool", bufs=2) as spool,
            tc.tile_pool(name="ps_s", bufs=2, space="PSUM") as ps_pool,
            tc.tile_pool(name="ps_o", bufs=1, space="PSUM") as po_pool,
            tc.tile_pool(name="ps_d", bufs=1, space="PSUM") as pd_pool,
        ):
            # constants built on fp32 scratch (memset/affine_select can't
            # write fp32r), then round-copied into typed tiles
            ones_sq = cpool.tile([P, P], BF16)
            nc.gpsimd.memset(ones_sq, 1.0)
            # pre-warm the PE during the input-DMA wait (~4us of dummy
            # matmuls) so the HAM clock gate is at 2.4 GHz for real work;
            # chunk 0's first denominator matmul clears the db bank anyway
            warm_db = pd_pool.tile([P, QCH], F32, tag="db", name="warm")
            for wi in range(72):
                nc.tensor.matmul(warm_db[:, ds(0, 64)], ones_sq,
                                 ones_sq[:, :64], start=True, stop=True)
            scratch = cpool.tile([P, 4 * P], F32)
            # pmask_d [128, (d+1)*128]: zeros prefix then upper-triangle
            # (keep where c - r - 128*d >= 0); multiplicative causal mask
            pmasks = []
            for d in range(4):
                w = (d + 1) * P
                nc.gpsimd.memset(scratch[:, :w], 1.0)
                nc.gpsimd.affine_select(
                    out=scratch[:, :w], in_=scratch[:, :w],
                    compare_op=mybir.AluOpType.is_ge, fill=0.0,
                    base=-d * P, pattern=[[1, w]], channel_multiplier=-1)
                pm = cpool.tile([P, w], BF16, name=f"pmask{d}")
                nc.vector.tensor_copy(pm, scratch[:, :w])
                pmasks.append(pm)

            # zero the two rotating S-PSUM slots once: narrowed diagonal
            # S-matmuls never write their masked column prefix, and
            # exp(uninitialized PSUM) can be inf (inf * 0 mask -> NaN)
            for si in range(2):
                s_init = ps_pool.tile([P, GROUP * QCH], F32, tag="s",
                                      name=f"s_init{si}")
                nc.vector.memset(s_init, 0.0)

            qT_sb = cpool.tile([P, NSEQ], BF16)
            kT_sb = cpool.tile([P, NSEQ], BF16)
            v_sb = cpool.tile([P, NSEQ], BF16)
            pw = NSEQ // NPIECE      # columns / k-rows per DMA piece
            for pi in range(NPIECE):
                sl = ds(pi * pw, pw)
                kq = nc.scalar if pi == 0 else nc.sync
                kq.dma_start(kT_sb[:, sl], kT[:, sl])
                nc.sync.dma_start(qT_sb[:, sl], qT[:, sl])
                vq = nc.gpsimd if pi == 0 else nc.sync
                vq.dma_start(
                    v_sb[:, sl].rearrange("p (j c) -> p j c", c=P),
                    v[sl, :].rearrange("(j p) c -> p j c", p=P))

            def emit_pv(job):
                # deferred PV + denominator matmuls for one group
                # (software pipelining: keeps the in-order PE queue from
                # head-of-line blocking on the exp/mask chain of the group)
                (t, j0, gn, nj, e_sb, o_ps, db_ps, den_blk,
                 den_first, den_last) = job
                for d in range(gn):
                    j = j0 + d
                    dd = j - 4 * t
                    off = max(dd, 0) * P
                    nc.tensor.matmul(
                        o_ps[:, ds(off, QCH - off)],
                        v_sb[:, ts(j, P)],
                        e_sb[:, ds(d * QCH + off, QCH - off)],
                        start=(j == 0), stop=(j == nj - 1))
                if den_blk is not None:
                    nc.tensor.matmul(db_ps, ones_sq, den_blk,
                                     start=den_first, stop=den_last)
                if j0 + gn == nj:      # last group: flush chunk outputs
                    out_sb = spool.tile([P, QCH], F32, tag="osb",
                                        name=f"osb{t}")
                    den_sb = spool.tile([1, QCH], F32, tag="den",
                                        name=f"den{t}")
                    if t == NCH - 1:   # tail: split copies across engines
                        nc.scalar.copy(out_sb, o_ps)
                        nc.vector.tensor_copy(den_sb, db_ps[0:1, :])
                    else:
                        nc.vector.tensor_copy(out_sb, o_ps)
                        nc.vector.tensor_copy(den_sb, db_ps[0:1, :])
                    nc.sync.dma_start(outT[:, ts(t, QCH)], out_sb)
                    nc.sync.dma_start(den[:, ts(t, QCH)], den_sb)

            pv_pending = None
            for t in range(NCH):
                nj = 4 * (t + 1)          # causal: k-tiles 0..4t+3
                q_sl = qT_sb[:, ts(t, QCH)]
                o_ps = po_pool.tile([P, QCH], F32, tag="o")
                db_ps = pd_pool.tile([P, QCH], F32, tag="db")
                den_carry = None
                den_count = 0

                groups = []
                j0 = 0
                while j0 < nj:
                    gn = min(GROUP, nj - j0)
                    groups.append((j0, gn))
                    j0 += gn

                for (j0, gn) in groups:
                    s_ps = ps_pool.tile([P, gn * QCH], F32, tag="s",
                                        padded_shape=[P, GROUP * QCH])
                    for d in range(gn):
                        j = j0 + d
                        dd = j - 4 * t
                        off = max(dd, 0) * P   # fully-masked column prefix
                        nc.tensor.matmul(
                            s_ps[:, ds(d * QCH + off, QCH - off)],
                            kT_sb[:, ts(j, P)], q_sl[:, ds(off, QCH - off)],
                            start=True, stop=True)
                    e_sb = epool.tile([P, gn * QCH], BF16, tag="e",
                                      padded_shape=[P, GROUP * QCH])
                    nc.scalar.activation(e_sb, s_ps, exp_fn, scale=SCALE)

                    # causal mask on diagonal tiles (j in [4t, 4t+4)):
                    # one multiply with the [zeros-prefix | triangle] mask
                    for d in range(gn):
                        j = j0 + d
                        dd = j - 4 * t
                        if dd >= 0:
                            reg = e_sb[:, ds(d * QCH, (dd + 1) * P)]
                            nc.vector.tensor_mul(reg, reg, pmasks[dd])

                    # denominator partials: sum blocks on DVE (bf16 2x
                    # adds), chaining across pairs of groups; one all-ones
                    # matmul per pair reduces over partitions into db
                    gidx = j0 // GROUP
                    chain = den_carry if gidx % 2 == 1 else None
                    if gn == 1 and chain is None:
                        den_blk = e_sb[:, :QCH]
                    else:
                        qacc = qpool.tile([P, QCH], BF16, tag="qacc")
                        first2 = (chain if chain is not None
                                  else e_sb[:, ts(1, QCH)])
                        nc.vector.tensor_add(qacc, e_sb[:, ts(0, QCH)],
                                             first2)
                        for d in range(1 if chain is not None else 2, gn):
                            nc.vector.tensor_add(qacc, qacc,
                                                 e_sb[:, ts(d, QCH)])
                        den_blk = qacc
                    if gidx % 2 == 0 and j0 + gn < nj:
                        den_carry = den_blk      # defer to next group
                        den_blk = None
                    else:
                        den_carry = None

                    if pv_pending is not None:
                        emit_pv(pv_pending)
                    den_first = den_blk is not None and den_count == 0
                    den_last = j0 + gn == nj
                    if den_blk is not None:
                        den_count += 1
                    pv_pending = (t, j0, gn, nj, e_sb, o_ps, db_ps, den_blk,
                                  den_first, den_last)

            emit_pv(pv_pending)

    nc.compile()
    return nc


def _get_nc():
    if not _nc_cache:
        _nc_cache.append(_build())
    return _nc_cache[0]


def kernel(query, key, value):
    B, H, W, C = query.shape
    CV = value.shape[-1]
    n = H * W
    q = (np.asarray(query, np.float32).reshape(B, n, C).transpose(0, 2, 1)
         .astype(ml_dtypes.bfloat16))
    q = np.ascontiguousarray(q)
    k = np.ascontiguousarray(
        np.asarray(key, np.float32).reshape(B, n, C).transpose(0, 2, 1)
        .astype(ml_dtypes.bfloat16))
    v = np.ascontiguousarray(
        np.asarray(value, np.float32).reshape(B, n, CV)
        .astype(ml_dtypes.bfloat16))

    nc = _get_nc()
    in_maps = [{"qT": q[b], "kT": k[b], "v": v[b]} for b in range(B)]
    res = run_bass_kernel_spmd(nc, in_maps, core_ids=list(range(N_CORES)))

    out = np.empty((B, n, CV), np.float32)
    for b in range(B):
        oT = res.results[b]["outT"]          # [128, 4096] unnormalized O^T
        dn = res.results[b]["den"]           # [1, 4096]
        out[b] = (oT / dn).T
    return out.reshape(B, H, W, CV)
